# revision 7
# baseline (speedup 1.0000x reference)
"""Trainium2 Bass kernel for nn_AttentionLayer (B=16, S=2048, H=1024, A=64).

Math (per batch b):
  Q = x Wq + bq ; K = x Wk + bk                      (S,A)
  E[s,t] = exp((Q[s].K[t])/sqrt(A)) * m[t]           (S,S)   mask folded in as
           an additive -BIG bias via an extra feature row in Q^T/K^T
  r[s] = sum_t E[s,t]  ;  attn[s,t] = E[s,t]/r[s]
  attn_mean[t] = (1/S) sum_s attn[s,t]               -> colsum, weights 1/(S*r)
  w[t] = sum_s m[s] attn[s,t]                        -> colsum, weights m/r
  out = (w^T (x Wv) + (sum_t w) bv) / (sum_s m + 1e-8)
      (sum_t w == sum_s m exactly, since attn rows sum to 1)

The big context matmul attn @ V is never materialized: just two weighted
column-sums of E on the TensorEngine, then a vector-matrix chain
w^T x (1,H) -> (w^T x) Wv (1,H).

Data-parallel over 8 cores: 2 batches per core, weights replicated.
All matmul operands are bf16 (1 cycle/row on PE); accumulation is f32.
"""
import os
import numpy as np
from contextlib import ExitStack

import concourse.bass as bass
import concourse.bacc as bacc
import concourse.tile as tile
from concourse import mybir
from concourse.bass_utils import run_bass_kernel_spmd
from concourse.masks import make_identity

F32 = mybir.dt.float32
BF16 = mybir.dt.bfloat16
I32 = mybir.dt.int32

B, S, H, A = 16, 2048, 1024, 64
N_CORES = 8
B_LOC = B // N_CORES          # 2 batches per core
NT = S // 128                 # 16 s-tiles
HK = H // 128                 # 8 h-tiles
SCALE = 8.0                   # sqrt(A)
BIG = 240000.0                # mask bias (pre exp-scale); exp(-BIG/SCALE) == 0
# Benchmark knob: repeat the whole per-core computation ITER times inside one
# NEFF. Grading path uses ITER=1 (identical graph to default).
ITER = int(os.environ.get("KERNEL_ITERS", "1"))


def build_nc():
    nc = bacc.Bacc("TRN2", target_bir_lowering=False, debug=False)

    x = nc.dram_tensor("x", [B_LOC, S, H], F32, kind="ExternalInput")
    mask = nc.dram_tensor("mask", [B_LOC, S], I32, kind="ExternalInput")
    Wq = nc.dram_tensor("Wq", [H, A], F32, kind="ExternalInput")
    bq = nc.dram_tensor("bq", [A], F32, kind="ExternalInput")
    Wk = nc.dram_tensor("Wk", [H, A], F32, kind="ExternalInput")
    bk = nc.dram_tensor("bk", [A], F32, kind="ExternalInput")
    Wv = nc.dram_tensor("Wv", [H, H], F32, kind="ExternalInput")
    bv = nc.dram_tensor("bv", [H], F32, kind="ExternalInput")
    out = nc.dram_tensor("out", [B_LOC, H], F32, kind="ExternalOutput")
    attn_mean = nc.dram_tensor("attn_mean", [B_LOC, S], F32, kind="ExternalOutput")

    with tile.TileContext(nc) as tc, ExitStack() as ctx:
        persist = ctx.enter_context(tc.tile_pool(name="persist", bufs=1))
        dpool = ctx.enter_context(tc.tile_pool(name="dscratch", bufs=2,
                                               space="DRAM"))

        ident = persist.tile([128, 128], BF16)
        make_identity(nc, ident)

        # weights, replicated layouts: partition = h % 128, free = (h//128, .)
        wqk = persist.tile([128, HK, 2 * A], BF16)
        nc.gpsimd.dma_start(wqk[:, :, 0:A],
                            Wq[:, :].rearrange("(k p) a -> p k a", p=128))
        nc.gpsimd.dma_start(wqk[:, :, A:2 * A],
                            Wk[:, :].rearrange("(k p) a -> p k a", p=128))
        wv_sb = persist.tile([128, HK, H], BF16)
        nc.gpsimd.dma_start(wv_sb[:], Wv[:, :].rearrange("(k p) h -> p k h", p=128))
        bq_col = persist.tile([A, 1], F32)
        nc.sync.dma_start(bq_col[:], bq[:])
        bk_col = persist.tile([A, 1], F32)
        nc.sync.dma_start(bk_col[:], bk[:])
        bv_row = persist.tile([1, H], F32)
        nc.sync.dma_start(bv_row[:], bv[:])

        # mask columns for both batches: (s%128, b, s//128)
        mcols_i = persist.tile([128, B_LOC, NT], I32)
        nc.sync.dma_start(mcols_i[:],
                          mask[:, :].rearrange("b (t p) -> p b t", p=128))
        mcols_f = persist.tile([128, B_LOC, NT], F32)
        nc.vector.tensor_copy(mcols_f[:], mcols_i[:])

        wx2 = persist.tile([128, HK, B_LOC], BF16)  # wx vectors as h-columns

        xpool = ctx.enter_context(tc.tile_pool(name="xp", bufs=2))
        qkpool = ctx.enter_context(tc.tile_pool(name="qkp", bufs=2))
        rowpool = ctx.enter_context(tc.tile_pool(name="rowp", bufs=2))

        for b in [bb for _ in range(ITER) for bb in range(B_LOC)]:
            x_sb = xpool.tile([128, NT, H], BF16)
            xr = x[b, :, :].rearrange("(t p) h -> p t h", p=128)
            for c in range(4):
                nc.gpsimd.dma_start(x_sb[:, c * 4:(c + 1) * 4, :],
                                    xr[:, c * 4:(c + 1) * 4, :])

            # per-batch mask row quantities (all partition-0 tiles)
            mrow_i = rowpool.tile([1, S], I32)
            nc.sync.dma_start(mrow_i[:], mask[b:b + 1, :])
            mrow_f = rowpool.tile([1, S], F32)
            nc.vector.tensor_copy(mrow_f[:], mrow_i[:])
            krow = rowpool.tile([1, S], F32)     # BIG*m - BIG
            nc.scalar.activation(krow[:], mrow_f[:],
                                 mybir.ActivationFunctionType.Copy,
                                 bias=-BIG, scale=BIG)
            nb_b = rowpool.tile([1, 1], F32)     # sum(m)
            nc.vector.reduce_sum(nb_b[:], mrow_f[:], axis=mybir.AxisListType.X)
            nbe_b = rowpool.tile([1, 1], F32)    # sum(m) + 1e-8
            nc.vector.tensor_scalar_add(nbe_b[:], nb_b[:], 1e-8)
            recnb_b = rowpool.tile([1, 1], F32)  # 1/(sum(m)+1e-8)
            nc.vector.reciprocal(recnb_b[:], nbe_b[:])

            QeT = qkpool.tile([A + 1, S], BF16)
            KeT = qkpool.tile([A + 1, S], BF16)
            nc.vector.memset(QeT[A:A + 1, :], 1.0)
            nc.gpsimd.dma_start(KeT[A:A + 1, :], krow[:])   # f32 -> bf16 cast

            # ---- phase A: transpose x, project Q^T and K^T (A+1 x S) ----
            with ExitStack() as actx:
                pT = actx.enter_context(
                    tc.tile_pool(name=f"pT{b}", bufs=2, space="PSUM"))
                pP = actx.enter_context(
                    tc.tile_pool(name=f"pP{b}", bufs=2, space="PSUM"))
                spool = actx.enter_context(tc.tile_pool(name=f"strip{b}", bufs=3))
                for c in range(4):              # s-chunks of 512
                    proj = pP.tile([128, 512], F32)
                    for k in range(HK):         # h-tiles of 128
                        tp = pT.tile([128, 512], BF16)
                        for j in range(4):
                            nc.tensor.transpose(
                                tp[:, j * 128:(j + 1) * 128],
                                x_sb[:, c * 4 + j, k * 128:(k + 1) * 128],
                                ident[:])
                        strip = spool.tile([128, 512], BF16)
                        nc.vector.tensor_copy(strip[:], tp[:])
                        nc.tensor.matmul(proj[:], wqk[:, k, :], strip[:],
                                         start=(k == 0), stop=(k == HK - 1))
                    sl = slice(c * 512, (c + 1) * 512)
                    nc.vector.tensor_scalar_add(QeT[0:A, sl], proj[0:A, :],
                                                bq_col[:])
                    nc.vector.tensor_scalar_add(KeT[0:A, sl], proj[A:128, :],
                                                bk_col[:])

            # ---- phase B: scores, exp, row-sums, weighted col-sums ----
            colrows = rowpool.tile([2, S], F32)
            w_cols = rowpool.tile([128, NT], BF16)
            scr_w = dpool.tile([2, S], F32)
            with ExitStack() as pctx:
                psS = pctx.enter_context(
                    tc.tile_pool(name=f"psS{b}", bufs=2, space="PSUM"))
                psC = pctx.enter_context(
                    tc.tile_pool(name=f"psC{b}", bufs=1, space="PSUM"))
                epool = pctx.enter_context(tc.tile_pool(name=f"E{b}", bufs=2))
                vpool = pctx.enter_context(tc.tile_pool(name=f"v{b}", bufs=4))
                colA = psC.tile([2, 1024], F32)
                colB = psC.tile([2, 1024], F32)
                for ti in range(NT):
                    E_t = epool.tile([128, S], BF16)
                    racc = vpool.tile([128, 2], F32)
                    for h2 in range(2):
                        sc = psS.tile([128, 1024], F32)
                        for cc in range(2):
                            tsl = slice((h2 * 2 + cc) * 512,
                                        (h2 * 2 + cc + 1) * 512)
                            nc.tensor.matmul(
                                sc[:, cc * 512:(cc + 1) * 512],
                                QeT[:, ti * 128:(ti + 1) * 128],
                                KeT[:, tsl])
                        nc.scalar.activation(
                            E_t[:, h2 * 1024:(h2 + 1) * 1024], sc[:],
                            mybir.ActivationFunctionType.Exp,
                            scale=1.0 / SCALE,
                            accum_out=racc[:, h2:h2 + 1])
                    rsum = vpool.tile([128, 1], F32)
                    nc.vector.tensor_add(rsum[:], racc[:, 0:1], racc[:, 1:2])
                    rec = vpool.tile([128, 1], F32)
                    nc.vector.reciprocal(rec[:], rsum[:])
                    W2 = vpool.tile([128, 2], BF16)
                    nc.vector.tensor_scalar_mul(W2[:, 0:1], rec[:], 1.0 / S)
                    nc.vector.tensor_mul(W2[:, 1:2], rec[:],
                                         mcols_f[:, b, ti:ti + 1])
                    for h2, colX in ((0, colA), (1, colB)):
                        for cc in range(2):
                            nc.tensor.matmul(
                                colX[:, cc * 512:(cc + 1) * 512],
                                W2[:],
                                E_t[:, (h2 * 2 + cc) * 512:
                                    (h2 * 2 + cc + 1) * 512],
                                start=(ti == 0), stop=(ti == NT - 1))

                # evacuate both column-sum rows together (partition base 0)
                nc.vector.tensor_copy(colrows[:, 0:1024], colA[:])
                nc.vector.tensor_copy(colrows[:, 1024:2048], colB[:])
            nc.sync.dma_start(attn_mean[b:b + 1, :], colrows[0:1, :])
            # w row -> DRAM -> back as 128xNT bf16 columns (s on partitions)
            nc.sync.dma_start(scr_w[:], colrows[:])
            nc.gpsimd.dma_start(w_cols[:],
                                scr_w[1, :].rearrange("(t p) -> p t", p=128))

            # ---- phase C: wx = w^T x (1,H);  out_b = (wx Wv + nb*bv)/nbe ----
            scr_wx = dpool.tile([1, H], F32)
            with ExitStack() as cctx:
                psW = cctx.enter_context(
                    tc.tile_pool(name=f"psW{b}", bufs=1, space="PSUM"))
                wx_ps = psW.tile([1, H], F32)
                for kt in range(NT):
                    for cc in range(2):
                        nc.tensor.matmul(
                            wx_ps[:, cc * 512:(cc + 1) * 512],
                            w_cols[:, kt:kt + 1],
                            x_sb[:, kt, cc * 512:(cc + 1) * 512],
                            start=(kt == 0), stop=(kt == NT - 1))
                wx_row = rowpool.tile([1, H], F32)
                nc.vector.tensor_copy(wx_row[:], wx_ps[:])
                nc.sync.dma_start(scr_wx[:], wx_row[:])
                nc.gpsimd.dma_start(wx2[:, :, b],
                                    scr_wx[0, :].rearrange("(k p) -> p k", p=128))

                fin_ps = psW.tile([1, H], F32)
                for kt in range(HK):
                    for cc in range(2):
                        nc.tensor.matmul(
                            fin_ps[:, cc * 512:(cc + 1) * 512],
                            wx2[:, kt, b:b + 1],
                            wv_sb[:, kt, cc * 512:(cc + 1) * 512],
                            start=(kt == 0), stop=(kt == HK - 1))
                bvs = rowpool.tile([1, H], F32)
                nc.vector.tensor_scalar_mul(bvs[:], bv_row[:], nb_b[:])
                fin_sb = rowpool.tile([1, H], F32)
                nc.vector.tensor_add(fin_sb[:], fin_ps[:], bvs[:])
                nc.vector.tensor_scalar_mul(fin_sb[:], fin_sb[:], recnb_b[:])
                nc.sync.dma_start(out[b:b + 1, :], fin_sb[:])

    nc.compile()
    return nc


_NC_CACHE = None


def _get_nc():
    global _NC_CACHE
    if _NC_CACHE is None:
        _NC_CACHE = build_nc()
    return _NC_CACHE


def kernel(x, mask, Wq, bq, Wk, bk, Wv, bv):
    x = np.ascontiguousarray(np.asarray(x, dtype=np.float32))
    mask = np.ascontiguousarray(np.asarray(mask, dtype=np.int32))
    Wq = np.ascontiguousarray(np.asarray(Wq, dtype=np.float32))
    bq = np.ascontiguousarray(np.asarray(bq, dtype=np.float32))
    Wk = np.ascontiguousarray(np.asarray(Wk, dtype=np.float32))
    bk = np.ascontiguousarray(np.asarray(bk, dtype=np.float32))
    Wv = np.ascontiguousarray(np.asarray(Wv, dtype=np.float32))
    bv = np.ascontiguousarray(np.asarray(bv, dtype=np.float32))

    nc = _get_nc()
    in_maps = []
    for i in range(N_CORES):
        sl = slice(i * B_LOC, (i + 1) * B_LOC)
        in_maps.append({
            "x": x[sl], "mask": mask[sl],
            "Wq": Wq, "bq": bq, "Wk": Wk, "bk": bk, "Wv": Wv, "bv": bv,
        })
    res = run_bass_kernel_spmd(nc, in_maps, core_ids=list(range(N_CORES)))
    outs = np.concatenate([res.results[i]["out"] for i in range(N_CORES)], axis=0)
    attns = np.concatenate([res.results[i]["attn_mean"] for i in range(N_CORES)],
                           axis=0)
    return outs.astype(np.float32), attns.astype(np.float32)
